# revision 5
# baseline (speedup 1.0000x reference)
"""Trainium2 Bass kernel for nn_AttentionMHA: 8-way tensor-parallel over heads.

Full attention prefill: B=1, S=2048, D=4096, H=32 Q-heads, KVH=8 KV-heads,
HD=128, causal (input_pos = arange(S)).

Per-core sharding (core c of 8): Q heads 4c..4c+3, KV head c, wo columns
512c..512(c+1).  Software-pipelined per token block j:
  QKV pass1(j) -> QKV pass2(j) -> attention(j-1) -> AllGather(j-1)
  -> RoPE/RMSNorm processing(j) -> output-projection(j-2)
so the PE never waits on the cross-engine RoPE/softmax chains and the
per-head AllGathers overlap with compute.  ATTN_DT selects the datatype of
the large matmuls (bf16 = full PE rate, f32r = half rate but ~10x more
accurate); RoPE/softmax/norm arithmetic stays fp32.

Perf notes (vs the first working version):
  * rsqrt for RMSNorm is computed as exp(-0.5*ln(ms+eps)) so the Act engine
    only ever needs one activation table (exp/ln/copy/square live in the
    same act_func_set) -- avoids ~1.3us ACT_TABLE_LOADs on the softmax path.
  * AllGather is per (block, head) with wo rows permuted head-major on the
    host, so collectives start as soon as each head finishes and the last
    collective before each output-projection tile is 4x smaller.
  * a dummy warm-up AllGather absorbs the cold-start cost of the CC rings.
  * initial weight/x DMAs are spread across engine queues.
  * attention matmuls for diagonal key chunks only cover the queries that
    can attend (query-range narrowing); masked-out columns are exp(-inf)=0
    anyway so the result is identical.
"""
import os
import sys

sys.path.insert(0, "/opt/trn_rl_repo")

import numpy as np
import ml_dtypes

import concourse.bass as bass
import concourse.bass_isa as bass_isa
import concourse.tile as tile
from concourse import bacc, mybir

f32 = mybir.dt.float32
f32r = mybir.dt.float32r
bf16 = mybir.dt.bfloat16
AF = mybir.ActivationFunctionType
ALU = mybir.AluOpType

B, S, D = 1, 2048, 4096
H, KVH, HD = 32, 8, 128
NH = 4            # q heads per core
TB = 512          # token block
NT = S // TB      # 4 token blocks
KC = D // 128     # 32 contraction chunks
NKT = S // 128    # 16 key chunks
EPS = 1e-5
SCALE = 1.0 / np.sqrt(HD)
NEG = -30000.0
N_CORES = 8

DT_BIG_NAME = os.environ.get("ATTN_DT", "bf16")

SWAP_MASK = list(range(16, 32)) + list(range(0, 16))


def build_nc(dt_name=None):
    dt_name = dt_name or DT_BIG_NAME
    dtb = bf16 if dt_name == "bf16" else f32r
    # DRAM dtype of the big inputs: bf16 tensors are host-converted; f32r is
    # a bitcast view of f32.
    dram_big = bf16 if dt_name == "bf16" else f32

    nc = bacc.Bacc("TRN2", target_bir_lowering=False, debug=False,
                   num_devices=N_CORES)

    XT = nc.dram_tensor("XT", [D, S], dram_big, kind="ExternalInput")
    WQ = nc.dram_tensor("WQ", [D, NH * HD], dram_big, kind="ExternalInput")
    WK = nc.dram_tensor("WK", [D, HD], dram_big, kind="ExternalInput")
    WV = nc.dram_tensor("WV", [D, HD], dram_big, kind="ExternalInput")
    WO = nc.dram_tensor("WO", [D, 512], dram_big, kind="ExternalInput")
    CC = nc.dram_tensor("CC", [HD, S], f32, kind="ExternalInput")
    SSI = nc.dram_tensor("SSI", [HD, S], f32, kind="ExternalInput")
    MASK = nc.dram_tensor("MASK", [128, 4 * TB], f32, kind="ExternalInput")
    IDM = nc.dram_tensor("IDM", [128, 128], f32, kind="ExternalInput")
    WQKC = nc.dram_tensor("WQKC", [128, 1], f32, kind="ExternalInput")
    ONESC = nc.dram_tensor("ONESC", [128, 1], f32, kind="ExternalInput")
    OUT = nc.dram_tensor("OUT", [S, 512], f32, kind="ExternalOutput")

    def big_view(t):
        ap = t.ap()
        return ap if dtb is bf16 else ap.bitcast(f32r)

    with tile.TileContext(nc) as tc, \
         nc.allow_low_precision(reason="intentional bf16/f32r operand rounding"):
        from contextlib import ExitStack
        with tc.tile_pool(name="dram", bufs=1, space="DRAM") as dram:
            # per-(block, head) AllGather staging: in [HD, TB] -> out
            # [8*HD, TB] (core-major rows within one head).
            y_ag = [[dram.tile([HD, TB], dram_big, name=f"yag{j}_{h}")
                     for h in range(NH)] for j in range(NT - 1)]
            y_full = [[dram.tile([N_CORES * HD, TB], dram_big,
                                 addr_space="Shared", name=f"yfull{j}_{h}")
                       for h in range(NH)] for j in range(NT - 1)]
            y_ag3 = [[dram.tile([HD, TB // 2], dram_big, name=f"yag3{g}_{h}")
                      for h in range(NH)] for g in range(2)]
            y_full3 = [[dram.tile([N_CORES * HD, TB // 2], dram_big,
                                  addr_space="Shared", name=f"yfull3{g}_{h}")
                        for h in range(NH)] for g in range(2)]
            warm_in = dram.tile([128, 8], dram_big, name="warmin")
            warm_out = dram.tile([N_CORES * 128, 8], dram_big,
                                 addr_space="Shared", name="warmout")
            ctx = ExitStack()
            with ctx:
                const = ctx.enter_context(tc.tile_pool(name="const", bufs=1))
                wqpool = ctx.enter_context(tc.tile_pool(name="wqpool", bufs=1))
                wopool = ctx.enter_context(tc.tile_pool(name="wopool", bufs=1))
                xtp = ctx.enter_context(tc.tile_pool(name="xtp", bufs=3 if dtb is bf16 else 4))
                qfp = ctx.enter_context(tc.tile_pool(name="qfp", bufs=4))
                resid = ctx.enter_context(tc.tile_pool(name="resid", bufs=1))
                tmp = ctx.enter_context(tc.tile_pool(name="tmp", bufs=2))
                smalls = ctx.enter_context(tc.tile_pool(name="smalls", bufs=1))
                expp = ctx.enter_context(tc.tile_pool(name="expp", bufs=4))
                ystp = ctx.enter_context(tc.tile_pool(name="ystp", bufs=1))
                ytp = ctx.enter_context(tc.tile_pool(name="ytp", bufs=3))
                outp = ctx.enter_context(tc.tile_pool(name="outp", bufs=1))
                # PSUM: proj(2) + scores(2) + y(2) + bcast(1) + dp/po(1) = 8
                proj = ctx.enter_context(
                    tc.tile_pool(name="proj", bufs=1, space="PSUM"))
                scoresp = ctx.enter_context(
                    tc.tile_pool(name="scoresp", bufs=2, space="PSUM"))
                ypp = ctx.enter_context(
                    tc.tile_pool(name="ypp", bufs=2, space="PSUM"))
                bcp = ctx.enter_context(
                    tc.tile_pool(name="bcp", bufs=1, space="PSUM"))
                dpp = ctx.enter_context(
                    tc.tile_pool(name="dpp", bufs=1, space="PSUM"))

                # ---- CC-ring warm-up: tiny AllGather nothing reads ----
                warm_t = const.tile([128, 8], dram_big)
                nc.gpsimd.memset(warm_t[:], 0.0)
                nc.gpsimd.dma_start(warm_in[:], warm_t[:])
                nc.gpsimd.collective_compute(
                    "AllGather", ALU.bypass,
                    replica_groups=[list(range(N_CORES))],
                    ins=[warm_in[:]], outs=[warm_out[:]])

                # ---- constants (gpsimd queue; needed ~40us in) ----
                cc_t = const.tile([HD, S], bf16)
                nc.gpsimd.dma_start(cc_t[:], CC.ap())
                ss_t = const.tile([HD, S], bf16)
                nc.gpsimd.dma_start(ss_t[:], SSI.ap())
                mask_t = const.tile([128, 4 * TB], bf16)
                nc.gpsimd.dma_start(mask_t[:], MASK.ap())
                id_t = const.tile([128, 128], f32)
                nc.sync.dma_start(id_t[:], IDM.ap())
                wqkc_t = const.tile([128, 1], f32)
                nc.sync.dma_start(wqkc_t[:], WQKC.ap())
                onesc_t = const.tile([128, 1], f32r)
                nc.sync.dma_start(onesc_t[:], ONESC.ap().bitcast(f32r))
                onesc_b = const.tile([128, 1], dtb)
                nc.vector.tensor_copy(onesc_b[:], onesc_t[:].bitcast(f32))
                eps_t = const.tile([1, 1], f32)
                nc.vector.memset(eps_t[:], EPS)

                wq_t = wqpool.tile([128, KC, NH * HD], dtb)
                wq_src3 = big_view(WQ).rearrange("(k p) n -> p k n", p=128)
                nc.sync.dma_start(wq_t[:, 0:8, :], wq_src3[:, 0:8, :])
                wo_t = wopool.tile([128, KC, 512], dtb)
                wkr_t = const.tile([128, KC, HD], dtb)
                wvr_t = const.tile([128, KC, HD], dtb)

                def late_weight_loads():
                    # scalar queue: wq k-chunk groups 1..3 (needed right
                    # after group 0); gpsimd queue: wk/wv (needed last).
                    for g in range(1, 4):
                        nc.scalar.dma_start(wq_t[:, g * 8:(g + 1) * 8, :],
                                            wq_src3[:, g * 8:(g + 1) * 8, :])
                    nc.gpsimd.dma_start(
                        wkr_t[:],
                        big_view(WK).rearrange("(k p) n -> p k n", p=128))
                    nc.gpsimd.dma_start(
                        wvr_t[:],
                        big_view(WV).rearrange("(k p) n -> p k n", p=128))

                kfin = resid.tile([128, S], dtb)
                vnat = resid.tile([128, NKT * 128], dtb)

                xt_srcp = big_view(XT).rearrange("(k p) t -> p k t", p=128)

                def process_qk(raw_psum, is_k, j):
                    """RoPE + RMSNorm from raw projection psum [128, TB]."""
                    qs = tmp.tile([128, TB], f32, tag="qs")
                    nc.scalar.copy(qs[:], raw_psum[:])
                    sq = tmp.tile([128, TB], f32r, tag="sq", bufs=1)
                    nc.scalar.square(sq[:], raw_psum[:])
                    rsp = bcp.tile([1, TB], f32, tag="bc")
                    nc.tensor.matmul(rsp[:], onesc_t[:], sq[:],
                                     start=True, stop=True)
                    # rr = (ms + eps)^-1/2 = exp(-0.5*ln(ms + eps)); ln/exp
                    # share an act table so no ACT_TABLE_LOADs are needed.
                    lnv = smalls.tile([1, TB], f32, tag="srt")
                    nc.scalar.activation(lnv[:], rsp[:], AF.Ln,
                                         bias=eps_t[:], scale=1.0 / HD)
                    rr = smalls.tile([1, TB], f32, tag="rr")
                    nc.scalar.activation(rr[:], lnv[:], AF.Exp, scale=-0.5)
                    bcb = tmp.tile([128, TB], f32, tag="bcb", bufs=1)
                    nc.gpsimd.partition_broadcast(bcb[:], rr[:])
                    tsw = tmp.tile([128, TB], f32, tag="tsw", bufs=1)
                    nc.vector.stream_shuffle(tsw[:], qs[:], SWAP_MASK)
                    t1 = tmp.tile([128, TB], f32, tag="t1")
                    nc.vector.tensor_tensor(
                        t1[:], qs[:], cc_t[:, j * TB:(j + 1) * TB], ALU.mult)
                    t2 = tmp.tile([128, TB], f32, tag="t2", bufs=1)
                    nc.vector.tensor_tensor(
                        t2[:], tsw[:], ss_t[:, j * TB:(j + 1) * TB], ALU.mult)
                    nc.vector.tensor_tensor(t1[:], t1[:], t2[:], ALU.add)
                    if is_k:
                        nc.vector.scalar_tensor_tensor(
                            kfin[:, j * TB:(j + 1) * TB], t1[:], wqkc_t[:],
                            bcb[:], ALU.mult, ALU.mult)
                        return None
                    qf = qfp.tile([128, TB], dtb, tag="qf")
                    nc.vector.tensor_tensor(qf[:], t1[:], bcb[:], ALU.mult)
                    return qf

                def load_xta(j, spread=False):
                    t0, t1 = j * TB, (j + 1) * TB
                    xta_a = xtp.tile([128, 16, TB], dtb, tag="xta")
                    xta_b = xtp.tile([128, 16, TB], dtb, tag="xta")
                    engs = [nc.sync] * 4
                    for g in range(4):
                        half = xta_a if g < 2 else xta_b
                        engs[g].dma_start(
                            half[:, (g % 2) * 8:(g % 2 + 1) * 8, :],
                            xt_srcp[:, g * 8:(g + 1) * 8, t0:t1])
                    return xta_a, xta_b

                def emit_qkv(j, xta_pre=None):
                    t0, t1 = j * TB, (j + 1) * TB
                    if dtb is bf16:
                        xta_a, xta_b = xta_pre or load_xta(j)
                    acc = []
                    for pidx in range(3):
                        pa = proj.tile([128, TB], f32, tag="pa")
                        pb = proj.tile([128, TB], f32, tag="pb")
                        acc.append((pa, pb))
                        for g in range(KC // 8):
                            if dtb is bf16:
                                half = xta_a if g < 2 else xta_b
                                xg = half[:, (g % 2) * 8:(g % 2 + 1) * 8, :]
                            else:
                                xg = xtp.tile([128, 8, TB], dtb, tag="xta")
                                nc.sync.dma_start(
                                    xg[:], xt_srcp[:, g * 8:(g + 1) * 8, t0:t1])
                            for kk in range(8):
                                k = g * 8 + kk
                                st, sp = (k == 0), (k == KC - 1)
                                if pidx < 2:
                                    h0 = 2 * pidx
                                    nc.tensor.matmul(
                                        pa[:],
                                        wq_t[:, k, h0 * 128:(h0 + 1) * 128],
                                        xg[:, kk, :], start=st, stop=sp)
                                    nc.tensor.matmul(
                                        pb[:],
                                        wq_t[:, k,
                                             (h0 + 1) * 128:(h0 + 2) * 128],
                                        xg[:, kk, :], start=st, stop=sp)
                                else:
                                    nc.tensor.matmul(
                                        pa[:], wkr_t[:, k, :], xg[:, kk, :],
                                        start=st, stop=sp)
                                    nc.tensor.matmul(
                                        pb[:], wvr_t[:, k, :], xg[:, kk, :],
                                        start=st, stop=sp)
                    # (q0, q1), (q2, q3), (k, v)
                    return [acc[0][0], acc[0][1], acc[1][0], acc[1][1],
                            acc[2][0], acc[2][1]]

                def emit_proc(j, ps):
                    pq0, pq1, pq2, pq3, pk, pv = ps
                    # release order must match proj-slot reuse order
                    q_tiles = [process_qk(pq0, False, j),
                               process_qk(pq1, False, j),
                               process_qk(pq2, False, j),
                               process_qk(pq3, False, j)]
                    process_qk(pk, True, j)
                    vt_s = tmp.tile([128, TB], f32, tag="vts", bufs=1)
                    nc.scalar.copy(vt_s[:], pv[:])
                    for ci in range(4):
                        pt = bcp.tile([128, 128], f32, tag="bc")
                        nc.tensor.transpose(
                            pt[:], vt_s[:, ci * 128:(ci + 1) * 128], id_t[:])
                        nc.vector.tensor_copy(
                            vnat[:, (4 * j + ci) * 128:(4 * j + ci + 1) * 128],
                            pt[:])
                    return q_tiles

                def emit_attention(j, q_tiles, filler=None, half=None):
                    """half=None: full TB block into y_ag[j].
                    half=0/1: qt sub-range of block NT-1 into y_ag3[half]."""
                    if half is None:
                        q0, qw = 0, TB
                        nchunks = 4 * (j + 1)
                        ag_in, ag_out = y_ag[j], y_full[j]
                    else:
                        q0, qw = half * (TB // 2), TB // 2
                        nchunks = 4 * j + 2 * (half + 1)
                        ag_in, ag_out = y_ag3[half], y_full3[half]
                    for h in range(NH):
                        qf = q_tiles[h]
                        yp = ypp.tile([128, qw], f32, tag="yp")
                        dps = dpp.tile([1, qw], f32, tag="dp")
                        for c in range(nchunks):
                            # diagonal chunks: only queries >= ci*128 (block
                            # relative) can attend; earlier columns stay 0.
                            ci = c - 4 * j
                            qlo = max(q0, ci * 128) if ci > 0 else q0
                            off = qlo - q0
                            sc = scoresp.tile([128, qw], f32, tag="sc")
                            nc.tensor.matmul(
                                sc[:, off:qw], kfin[:, c * 128:(c + 1) * 128],
                                qf[:, qlo:q0 + qw], start=True, stop=True)
                            if ci >= 0:
                                nc.vector.tensor_tensor(
                                    sc[:, off:qw], sc[:, off:qw],
                                    mask_t[:, ci * TB + qlo:
                                           ci * TB + q0 + qw], ALU.add)
                            ex = expp.tile([128, qw], dtb, tag="ex")
                            nc.scalar.activation(ex[:, off:qw], sc[:, off:qw],
                                                 AF.Exp, scale=SCALE)
                            nc.tensor.matmul(
                                yp[:, off:qw],
                                vnat[:, c * 128:(c + 1) * 128], ex[:, off:qw],
                                start=(c == 0), stop=(c == nchunks - 1))
                            nc.tensor.matmul(
                                dps[:, off:qw], onesc_b[:], ex[:, off:qw],
                                start=(c == 0), stop=(c == nchunks - 1))
                        dcp = smalls.tile([1, qw], f32, tag="dcp")
                        nc.scalar.copy(dcp[:], dps[:])
                        drec = smalls.tile([1, qw], f32, tag="drec")
                        nc.vector.reciprocal_approx_fast(drec[:], dcp[:])
                        dbc = tmp.tile([128, qw], f32, tag="dbc", bufs=1)
                        nc.gpsimd.partition_broadcast(dbc[:], drec[:])
                        yst = ystp.tile([128, qw], dram_big, tag="yst")
                        nc.vector.tensor_tensor(yst[:], yp[:], dbc[:],
                                                ALU.mult)
                        nc.sync.dma_start(ag_in[h][:], yst[:])
                        # gather this head right away so collectives overlap
                        # the remaining heads / next block's compute
                        nc.gpsimd.collective_compute(
                            "AllGather", ALU.bypass,
                            replica_groups=[list(range(N_CORES))],
                            ins=[ag_in[h][:]], outs=[ag_out[h][:]])
                        if filler is not None:
                            filler(h)

                def emit_wo_tile(j, ti):
                    if j == NT - 1:
                        bufs = y_full3[ti // 2]
                        toff = (ti % 2) * 128
                    else:
                        bufs = y_full[j]
                        toff = ti * 128
                    po = ypp.tile([128, 512], f32, tag="yp")
                    for g in range(4):
                        srcd = bufs[g][:]
                        if dtb is f32r:
                            srcd = srcd.bitcast(f32r)
                        srcd = srcd.rearrange("(k p) t -> p k t", p=128)
                        yt = ytp.tile([128, 8, 128], dtb, tag="yt")
                        eng = nc.sync if g % 2 == 0 else nc.gpsimd
                        eng.dma_start(yt[:], srcd[:, :, toff:toff + 128])
                        for kk in range(8):
                            k = g * 8 + kk
                            nc.tensor.matmul(po[:], yt[:, kk, :],
                                             wo_t[:, k, :],
                                             start=(k == 0),
                                             stop=(k == KC - 1))
                    ot = outp.tile([128, 512], f32, tag="ot")
                    nc.vector.tensor_copy(ot[:], po[:])
                    t = 4 * j + ti
                    nc.sync.dma_start(OUT.ap()[t * 128:(t + 1) * 128, :],
                                      ot[:])

                def emit_wo(j):
                    for ti in range(4):
                        emit_wo_tile(j, ti)

                # ---- software-pipelined emission ----
                prev_q = None
                for j in range(NT):
                    if j == 0 and dtb is bf16:
                        xta0 = load_xta(0, spread=True)
                        late_weight_loads()
                        ps = emit_qkv(j, xta_pre=xta0)
                    else:
                        ps = emit_qkv(j)
                    if j == 0 and dtb is not bf16:
                        late_weight_loads()
                    if j == 0:
                        wo_src3 = big_view(WO).rearrange(
                            "(k p) n -> p k n", p=128)
                        for g in range(4):
                            eng = nc.gpsimd if g % 2 == 0 else nc.sync
                            eng.dma_start(
                                wo_t[:, g * 8:(g + 1) * 8, :],
                                wo_src3[:, g * 8:(g + 1) * 8, :])
                    new_q = emit_proc(j, ps)
                    if j >= 1:
                        emit_attention(j - 1, prev_q)
                    prev_q = new_q
                    if j >= 2:
                        emit_wo(j - 2)
                emit_attention(NT - 1, prev_q, half=0,
                               filler=lambda h: emit_wo_tile(NT - 2, h))
                emit_attention(NT - 1, prev_q, half=1,
                               filler=lambda h: emit_wo_tile(NT - 1, h // 2)
                               if h % 2 else None)
                emit_wo_tile(NT - 1, 2)
                emit_wo_tile(NT - 1, 3)

    nc.compile()
    return nc


_PERM = None


def _perm():
    """Within-head permutation: quadrant q holds pairs 16q..16q+15 as
    16 real rows then 16 imag rows (stream_shuffle swaps within quadrants)."""
    global _PERM
    if _PERM is None:
        p = np.zeros(HD, dtype=np.int64)
        for q in range(4):
            for jj in range(16):
                p[32 * q + jj] = 2 * (16 * q + jj)
                p[32 * q + 16 + jj] = 2 * (16 * q + jj) + 1
        _PERM = p
    return _PERM


def make_inputs(x, freqs_cos, freqs_sin, wq, wk, wv, wo, q_norm_w, k_norm_w,
                dt_name=None):
    dt_name = dt_name or DT_BIG_NAME
    np_big = ml_dtypes.bfloat16 if dt_name == "bf16" else np.float32
    perm = _perm()
    xT = np.ascontiguousarray(x.reshape(S, D).T).astype(np_big)
    cosT = np.ascontiguousarray(freqs_cos.T)  # [64, S]
    sinT = np.ascontiguousarray(freqs_sin.T)
    cc = np.empty((HD, S), dtype=np.float32)
    ssg = np.empty((HD, S), dtype=np.float32)
    for q in range(4):
        cc[32 * q:32 * q + 16] = cosT[16 * q:16 * q + 16]
        cc[32 * q + 16:32 * q + 32] = cosT[16 * q:16 * q + 16]
        ssg[32 * q:32 * q + 16] = -sinT[16 * q:16 * q + 16]
        ssg[32 * q + 16:32 * q + 32] = sinT[16 * q:16 * q + 16]
    mask = np.empty((128, 4 * TB), dtype=np.float32)
    qt = np.arange(TB)
    for ci in range(4):
        kt = 128 * ci + np.arange(128)
        mask[:, ci * TB:(ci + 1) * TB] = np.where(
            kt[:, None] <= qt[None, :], 0.0, NEG).astype(np.float32)
    wqk = (q_norm_w * k_norm_w)[perm].reshape(HD, 1).astype(np.float32)
    common = dict(
        XT=xT, CC=cc, SSI=ssg, MASK=mask,
        IDM=np.eye(128, dtype=np.float32), WQKC=wqk,
        ONESC=np.ones((128, 1), dtype=np.float32),
    )
    in_maps = []
    for c in range(N_CORES):
        wq_c = wq[:, c * NH * HD:(c + 1) * NH * HD].reshape(D, NH, HD)
        wq_c = np.ascontiguousarray(wq_c[:, :, perm].reshape(D, NH * HD))
        wk_c = np.ascontiguousarray(wk[:, c * HD:(c + 1) * HD][:, perm])
        wv_c = np.ascontiguousarray(wv[:, c * HD:(c + 1) * HD])
        wo_c = wo[:, c * 512:(c + 1) * 512]
        # AllGather is per head: gathered y rows are (head, core, hd) so
        # permute wo rows from (core, head, hd) to match.
        wo_c = np.ascontiguousarray(
            wo_c.reshape(N_CORES, NH, HD, 512).transpose(1, 0, 2, 3)
            .reshape(D, 512))
        in_maps.append(dict(
            common, WQ=wq_c.astype(np_big), WK=wk_c.astype(np_big),
            WV=wv_c.astype(np_big), WO=wo_c.astype(np_big)))
    return in_maps


_NC = None


def get_nc():
    global _NC
    if _NC is None:
        _NC = build_nc()
    return _NC


def kernel(x, freqs_cos, freqs_sin, input_pos, wq, wk, wv, wo,
           q_norm_w, k_norm_w, k_cache, v_cache):
    from concourse.bass_utils import run_bass_kernel_spmd
    nc = get_nc()
    in_maps = make_inputs(np.asarray(x), np.asarray(freqs_cos),
                          np.asarray(freqs_sin), np.asarray(wq),
                          np.asarray(wk), np.asarray(wv), np.asarray(wo),
                          np.asarray(q_norm_w), np.asarray(k_norm_w))
    res = run_bass_kernel_spmd(nc, in_maps, core_ids=list(range(N_CORES)))
    out = np.concatenate([res.results[c]["OUT"] for c in range(N_CORES)],
                         axis=1)
    return out.reshape(B, S, D).astype(np.float32)
